# revision 1
# baseline (speedup 1.0000x reference)
"""Trainium2 Bass kernel for nn_BatchedModelManifoldGeodesicFlow.

Math (validated vs reference):
  G = J^T J + eps*I is symmetric => the Christoffel contraction collapses:
    einsum('bijk,bj,bk->bi', Gamma, v, v) = 0.5 * einsum('bijk,bj,bk->bi', dG, v, v)
  With f = tanh(x@W1+b1)@W2+b2 (J = W2^T diag(d1) W1^T, per-output Hessians
  H_o = W1 diag(W2[:,o]*d2) W1^T where d1 = 1-s^2, d2 = -2*s*d1, s = tanh(z)):

    T_i := sum_jk dG[i,j,k] v_j v_k = (W1 @ g)_i,
       g = d2*p*(S@(d1*p)) + d1*(S@(d2*p*p)),  p = W1^T v,  S = W2@W2^T
    ||dG||_F^2 = 2*(<G1,G2> + sum_{o,o'} Y[:,(o,o')].Y[:,(o',o)])
       E = d1[:,None]*W2, C = d2[:,None]*W2, K = W1^T W1, K2 = K*K
       F = K@E, G1 = E^T F, G2 = C^T (K2@C), Y[:, o*O+o'] = W1@(C[:,o']*F[:,o])
    a = -0.5*T/((||dG||_F+1e-6)*(||v||+1e-6));  out = concat(v, a - 0.1*dev)

Sharding: pure batch parallelism, B=32 over 8 cores (4 samples/core),
params replicated. Layout on-chip is feature-major: [d or h partitions,
batch columns]; per-sample O-space tensors are batched into 40/400-wide
free dims so each weight matrix is loaded into the PE once per use.
"""

import os
import sys

if "/opt/trn_rl_repo" not in sys.path:
    sys.path.insert(0, "/opt/trn_rl_repo")

import numpy as np

B, D, H, O = 32, 128, 256, 10
NCORES = 8
BC = B // NCORES  # 4 samples per core
OO = O * O  # 100

_PROGRAM = None
ZERO_STRIDE_MX = os.environ.get("ZERO_STRIDE_MX", "1") == "1"
PSUM_DIRECT = os.environ.get("PSUM_DIRECT", "0") == "1"
# debug bisection stage: build only a prefix of the graph and dump that
# stage's tile through the output
_STAGES = ["xf", "consts", "sp", "q", "fkc", "ga", "mx", "y", "tb", "red", "full"]
KSTAGE = os.environ.get("KSTAGE", "full")


def _build_program():
    import concourse.bass as bass
    import concourse.bacc as bacc
    import concourse.tile as tile
    from concourse import mybir

    f32 = mybir.dt.float32
    mult = mybir.AluOpType.mult
    add = mybir.AluOpType.add
    AF = mybir.ActivationFunctionType

    stage_i = _STAGES.index(KSTAGE)

    def ge(s):  # build sections up to and including KSTAGE
        return stage_i >= _STAGES.index(s)

    from concourse.masks import make_identity

    nc = bacc.Bacc(None)
    xc_d = nc.declare_dram_parameter("xc", [4 * BC, D], f32, isOutput=False)
    aux_d = nc.declare_dram_parameter("aux", [128, 23], f32, isOutput=False)
    w1_d = nc.declare_dram_parameter("W1", [D, H], f32, isOutput=False)
    acc_d = nc.declare_dram_parameter("acc", [BC, D], f32, isOutput=True)

    with tile.TileContext(nc) as tc:
        with (
            tc.tile_pool(name="const", bufs=1) as const,
            tc.tile_pool(name="work", bufs=1) as work,
            tc.tile_pool(name="ps", bufs=1, space="PSUM") as ps,
        ):
            # ---------------- loads ----------------
            # aux host-packed: cols 0:2 b1 (h-chunked), 2:22 W2 (cols hc*10+o),
            # 22:23 t replicated to all partitions
            xc_sb = const.tile([4 * BC, D], f32)
            nc.sync.dma_start(out=xc_sb, in_=xc_d[:])
            aux_sb = const.tile([128, 23], f32)
            nc.sync.dma_start(out=aux_sb, in_=aux_d[:])
            w1_sb = const.tile([D, H], f32)
            nc.sync.dma_start(out=w1_sb, in_=w1_d[:])
            b1_sb = aux_sb[:, 0:2]
            w2_sb = aux_sb[:, 2 : 2 + 2 * O]
            tt = aux_sb[:, 22:23]

            id_sb = const.tile([128, 128], f32)
            make_identity(nc, id_sb[:])


            ones_c = const.tile([128, 1], f32)
            nc.vector.memset(ones_c, 1.0)
            devneg = const.tile([BC, D], f32)
            nc.scalar.mul(devneg, xc_sb[0:BC, :], -0.1)
            twos_c = const.tile([128, 1], f32)
            nc.vector.memset(twos_c, 2.0)

            target = None  # [pt, BC] tile dumped through acc when KSTAGE != full

            # ---------------- input transpose: [16,128] -> [128,16] ----------------
            ps_xt = ps.tile([128, 4 * BC], f32, tag="ps", bufs=2)
            nc.tensor.transpose(ps_xt, xc_sb, id_sb[: 4 * BC, : 4 * BC])
            xt = work.tile([128, 4 * BC], f32)
            nc.vector.tensor_copy(xt, ps_xt)
            devt = xt[:, 0:BC]
            x0t = xt[:, BC : 2 * BC]
            x1t = xt[:, 2 * BC : 3 * BC]
            vt = xt[:, 3 * BC : 4 * BC]

            # window = 4*t*(1-t)
            omt = work.tile([128, 1], f32)
            nc.vector.tensor_scalar(omt, tt, -1.0, 1.0, mult, add)
            wrow = work.tile([128, 1], f32)
            nc.vector.scalar_tensor_tensor(wrow, tt, 4.0, omt, mult, mult)

            # ---------------- x = x0 + t*(x1-x0) + window*dev ----------------
            dx = work.tile([128, BC], f32)
            nc.vector.tensor_sub(dx, x1t, x0t)
            xa = work.tile([128, BC], f32)
            nc.vector.scalar_tensor_tensor(xa, dx, tt[:, 0:1], x0t, mult, add)
            zp_rhs = work.tile([128, 2 * BC], f32)  # cols 0:BC = x, BC:2BC = v
            xf = zp_rhs[:, 0:BC]
            nc.vector.scalar_tensor_tensor(xf, devt, wrow[:, 0:1], xa, mult, add)
            nc.vector.tensor_copy(zp_rhs[:, BC : 2 * BC], vt)
            if KSTAGE == "xf":
                target = xf

            if ge("consts"):
                if KSTAGE == "consts":
                    target = work.tile([128, BC], f32, name="dbg_consts")
                    nc.vector.tensor_copy(target, xt[:, 0:BC])

            if ge("sp"):
                # ---------------- z = W1^T x (+b1, tanh), p = W1^T v ----------------
                s_act = work.tile([128, 2 * BC], f32)  # tanh(z), cols hc*BC+b
                p_t = work.tile([128, 2 * BC], f32)
                for hc in range(2):
                    ps_zp = ps.tile([128, 2 * BC], f32, tag="ps", bufs=2)
                    nc.tensor.matmul(ps_zp, w1_sb[:, hc * 128 : (hc + 1) * 128], zp_rhs, start=True, stop=True)
                    nc.scalar.activation(
                        s_act[:, hc * BC : (hc + 1) * BC], ps_zp[:, 0:BC], AF.Tanh, bias=b1_sb[:, hc : hc + 1]
                    )
                    nc.vector.tensor_copy(p_t[:, hc * BC : (hc + 1) * BC], ps_zp[:, BC : 2 * BC])

                # ---------------- d1 = 1-s^2, d2 = -2*s*d1 (per chunk) ----------------
                d1 = work.tile([128, 2 * BC], f32)
                d2 = work.tile([128, 2 * BC], f32)
                for hc in range(2):
                    hsl = slice(hc * BC, (hc + 1) * BC)
                    nc.vector.tensor_mul(d1[:, hsl], s_act[:, hsl], s_act[:, hsl])
                    nc.vector.tensor_scalar(d1[:, hsl], d1[:, hsl], -1.0, 1.0, mult, add)
                    nc.vector.scalar_tensor_tensor(d2[:, hsl], s_act[:, hsl], -2.0, d1[:, hsl], mult, mult)
                # ---------------- W1^T / W2^T (after tanh so ACT queue stays clear) ----------------
                w1t_sb = []
                for hc in range(2):
                    ps_w1t = ps.tile([128, 128], f32, tag="ps", bufs=2)
                    nc.tensor.transpose(ps_w1t, w1_sb[:, hc * 128 : (hc + 1) * 128], id_sb)
                    w1t_t = const.tile([128, 128], f32, tag=f"w1t{hc}")
                    nc.scalar.copy(w1t_t, ps_w1t)
                    w1t_sb.append(w1t_t)
                w2t_sb = const.tile([O, H], f32)
                for hc in range(2):
                    ps_w2t = ps.tile([O, 128], f32, tag="ps", bufs=2)
                    nc.tensor.transpose(ps_w2t, w2_sb[:, hc * O : (hc + 1) * O], id_sb)
                    nc.scalar.copy(w2t_sb[:, hc * 128 : (hc + 1) * 128], ps_w2t)

            if ge("consts"):
                # ---------------- K = W1^T W1 (programmed after z so z preempts PE) ----------------
                k_sb = []
                k2_sb = []
                ps_k_list = []
                for mc in range(2):
                    ps_k = ps.tile([128, H], f32, tag="Y", bufs=2)
                    for q4 in range(4):
                        nc.tensor.matmul(
                            ps_k[:, q4 * 64 : (q4 + 1) * 64],
                            w1_sb[:, mc * 128 : (mc + 1) * 128],
                            w1_sb[:, q4 * 64 : (q4 + 1) * 64],
                            start=True,
                            stop=True,
                            skip_group_check=True,
                        )
                    ps_k_list.append(ps_k)
                    k_sb.append(const.tile([128, H], f32, tag=f"k{mc}", name=f"k_t{mc}"))
                    k2_sb.append(const.tile([128, H], f32, tag=f"k2{mc}", name=f"k2_t{mc}"))
                nc.scalar.copy(k_sb[0], ps_k_list[0])
                nc.gpsimd.tensor_mul(k2_sb[0], k_sb[0], k_sb[0])

                if KSTAGE == "sp":
                    target = work.tile([128, BC], f32, name="dbg_sp")
                    nc.vector.tensor_copy(target, d2[:, 0:BC])

            if ge("q"):
                # ---------------- q = S @ [d1*p | d2*p*p] ----------------
                d2p = work.tile([128, 2 * BC], f32)
                nc.vector.tensor_mul(d2p, d2, p_t)
                qrhs = work.tile([128, 4 * BC], f32)  # cols: [d1p (2*BC) | d2pp (2*BC)]
                nc.vector.tensor_mul(qrhs[:, 0 : 2 * BC], d1, p_t)
                nc.vector.tensor_mul(qrhs[:, 2 * BC : 4 * BC], d2p, p_t)

                # q = S @ r = W2 @ (W2^T @ r): two thin matmuls, no S materialization
                qv = qrhs[:].rearrange("p (g c b) -> p c g b", g=2, c=2, b=BC)
                ps_u = ps.tile([O, 2 * BC], f32, tag="q", bufs=3)
                for kc in range(2):
                    nc.tensor.matmul(
                        ps_u,
                        w2_sb[:, kc * O : (kc + 1) * O],
                        qv[:, kc],
                        start=(kc == 0),
                        stop=(kc == 1),
                    )
                u_sb = work.tile([O, 2 * BC], f32)
                nc.vector.tensor_copy(u_sb, ps_u)
                ps_q = []
                for mc in range(2):
                    ps_qm = ps.tile([128, 2 * BC], f32, tag="q", bufs=3)
                    nc.tensor.matmul(
                        ps_qm, w2t_sb[:, mc * 128 : (mc + 1) * 128], u_sb, start=True, stop=True
                    )
                    ps_q.append(ps_qm)

                # ---------------- g = d2*p*q1 + d1*q2 ; T = W1 @ g ----------------
                g_t = work.tile([128, 2 * BC], f32)
                tmp_dq = work.tile([128, 2 * BC], f32)
                for hc in range(2):
                    nc.vector.tensor_mul(g_t[:, hc * BC : (hc + 1) * BC], d2p[:, hc * BC : (hc + 1) * BC], ps_q[hc][:, 0:BC])
                    nc.vector.tensor_mul(tmp_dq[:, hc * BC : (hc + 1) * BC], d1[:, hc * BC : (hc + 1) * BC], ps_q[hc][:, BC : 2 * BC])
                nc.vector.tensor_add(g_t, g_t, tmp_dq)
                ps_T = ps.tile([128, BC], f32, tag="red", bufs=1)
                for hc in range(2):
                    nc.tensor.matmul(ps_T, w1t_sb[hc], g_t[:, hc * BC : (hc + 1) * BC], start=(hc == 0), stop=(hc == 1))
                ps_T_sb = work.tile([128, BC], f32)
                nc.vector.tensor_copy(ps_T_sb, ps_T)
                if KSTAGE == "q":
                    target = ps_T_sb

            if ge("fkc"):
                # ---------------- F = K@E, KC = K2@C (batched over samples) ----------------
                e_all = []
                c_all = []
                for hc in range(2):
                    e_t = work.tile([128, BC * O], f32, tag=f"e{hc}")
                    c_t = work.tile([128, BC * O], f32, tag=f"c{hc}")
                    w2_blk = w2_sb[:, hc * O : (hc + 1) * O]
                    w2_view = bass.AP(
                        tensor=w2_blk.tensor, offset=w2_blk.offset,
                        ap=[w2_blk.ap[0], [0, BC], list(w2_blk.ap[1])],
                    )
                    d1_blk = d1[:, hc * BC : (hc + 1) * BC]
                    d1_view = bass.AP(
                        tensor=d1_blk.tensor, offset=d1_blk.offset,
                        ap=[d1_blk.ap[0], list(d1_blk.ap[1]), [0, O]],
                    )
                    d2_blk = d2[:, hc * BC : (hc + 1) * BC]
                    d2_view = bass.AP(
                        tensor=d2_blk.tensor, offset=d2_blk.offset,
                        ap=[d2_blk.ap[0], list(d2_blk.ap[1]), [0, O]],
                    )
                    nc.vector.tensor_tensor(
                        e_t[:].rearrange("p (b o) -> p b o", b=BC), w2_view, d1_view, mult
                    )
                    nc.gpsimd.tensor_tensor(
                        c_t[:].rearrange("p (b o) -> p b o", b=BC), w2_view, d2_view, mult
                    )
                    e_all.append(e_t)
                    c_all.append(c_t)

                # deferred off the tanh->E/C critical path
                nc.vector.tensor_copy(k_sb[1], ps_k_list[1])
                nc.gpsimd.tensor_mul(k2_sb[1], k_sb[1], k_sb[1])

                f_sb = work.tile([128, 2 * BC * O], f32)  # cols mc*40 + (b*10+o)
                kc_sb = work.tile([128, 2 * BC * O], f32)
                for mc in range(2):
                    ps_f = ps.tile([128, BC * O], f32, tag="ps", bufs=2)
                    for kc in range(2):
                        nc.tensor.matmul(
                            ps_f, k_sb[kc][:, mc * 128 : (mc + 1) * 128], e_all[kc], start=(kc == 0), stop=(kc == 1)
                        )
                    nc.vector.tensor_copy(f_sb[:, mc * BC * O : (mc + 1) * BC * O], ps_f)
                    ps_kc = ps.tile([128, BC * O], f32, tag="ps", bufs=2)
                    for kc in range(2):
                        nc.tensor.matmul(
                            ps_kc, k2_sb[kc][:, mc * 128 : (mc + 1) * 128], c_all[kc], start=(kc == 0), stop=(kc == 1)
                        )
                    nc.scalar.copy(kc_sb[:, mc * BC * O : (mc + 1) * BC * O], ps_kc)
                if KSTAGE == "fkc":
                    target = work.tile([128, BC], f32, name="dbg_fkc")
                    nc.vector.tensor_copy(target, f_sb[:, 0:BC])

            if ge("ga"):
                # ------------ G1_b = E_b^T F_b, G2_b = C_b^T KC_b ; termA_b = <G1_b,G2_b> ------------
                GA_LVL = int(os.environ.get("GA_LVL", "4"))
                ta = work.tile([O, BC], f32)
                if GA_LVL < 4:
                    nc.vector.memset(ta, 0.0)
                g1s_sb = work.tile([O, BC * 2 * O], f32)
                junk_a = work.tile([O, BC * O], f32)
                for b in range(BC):
                    ps_g = ps.tile([O, 2 * O], f32, tag="q", bufs=3)
                    for hc in range(2):
                        nc.tensor.matmul(
                            ps_g[:, 0:O],
                            e_all[hc][:, b * O : (b + 1) * O],
                            f_sb[:, hc * BC * O + b * O : hc * BC * O + (b + 1) * O],
                            start=(hc == 0),
                            stop=(hc == 1),
                            skip_group_check=True,
                        )
                    for hc in range(2):
                        nc.tensor.matmul(
                            ps_g[:, O : 2 * O],
                            c_all[hc][:, b * O : (b + 1) * O],
                            kc_sb[:, hc * BC * O + b * O : hc * BC * O + (b + 1) * O],
                            start=(hc == 0),
                            stop=(hc == 1),
                            skip_group_check=True,
                        )
                    g12 = g1s_sb[:, b * 2 * O : (b + 1) * 2 * O]
                    nc.vector.tensor_copy(g12, ps_g)
                    ja = junk_a[:, b * O : (b + 1) * O]
                    nc.vector.scalar_tensor_tensor(
                        ja,
                        g12[:, 0:O],
                        1.0,
                        g12[:, O : 2 * O],
                        mult,
                        mult,
                        accum_out=ta[:, b : b + 1],
                    )
                if KSTAGE == "ga":
                    target = work.tile([128, BC], f32, name="dbg_ga")
                    nc.vector.memset(target, 0.0)
                    nc.vector.tensor_copy(target[0:O, :], ta)

            if ge("mx"):
                # ---------------- Mx build ----------------
                mx = [
                    work.tile([128, BC * OO], f32, tag="mx0", name="mx_t0"),
                    work.tile([128, BC * OO], f32, tag="mx1", name="mx_t1"),
                ]
                # b-major order: Y half hf needs (b=2hf, 2hf+1) x both hc first
                for b in range(BC):
                    for hc in range(2):
                        mx_t = mx[hc]
                        c_blk = c_all[hc][:, b * O : (b + 1) * O]
                        if ZERO_STRIDE_MX:
                            f_blk = f_sb[:, hc * BC * O + b * O : hc * BC * O + (b + 1) * O]
                            c_view = bass.AP(
                                tensor=c_blk.tensor, offset=c_blk.offset, ap=[c_blk.ap[0], [0, O], list(c_blk.ap[1])]
                            )
                            f_view = bass.AP(
                                tensor=f_blk.tensor, offset=f_blk.offset, ap=[f_blk.ap[0], list(f_blk.ap[1]), [0, O]]
                            )
                            nc.gpsimd.tensor_tensor(
                                mx_t[:, b * OO : (b + 1) * OO].rearrange("p (a c) -> p a c", a=O),
                                f_view,
                                c_view,
                                mult,
                            )
                        else:
                            for o in range(O):
                                nc.vector.tensor_scalar_mul(
                                    mx_t[:, b * OO + o * O : b * OO + (o + 1) * O],
                                    c_blk,
                                    f_sb[:, hc * BC * O + b * O + o : hc * BC * O + b * O + o + 1],
                                )
                if KSTAGE == "mx":
                    target = work.tile([128, BC], f32, name="dbg_mx")
                    nc.vector.tensor_copy(target, mx[0][:, 0:BC])

            if ge("y"):
                # ---------------- Y = W1 @ Mx  (two sample-halves) + termB fused ----------------
                half = BC // 2 * OO  # 200 cols per half
                tb = work.tile([128, BC], f32)
                junk_b = work.tile([128, BC * OO], f32)
                y_sb = work.tile([128, BC * OO], f32)
                for hf in range(2):
                    ps_y = ps.tile([128, half], f32, tag="Y", bufs=2)
                    for hc in range(2):
                        nc.tensor.matmul(
                            ps_y,
                            w1t_sb[hc],
                            mx[hc][:, hf * half : (hf + 1) * half],
                            start=(hc == 0),
                            stop=(hc == 1),
                        )
                    for bi in range(2):
                        b = hf * 2 + bi
                        blk_sb = y_sb[:, b * OO : (b + 1) * OO]
                        if b == 3:
                            nc.scalar.copy(blk_sb, ps_y[:, bi * OO : (bi + 1) * OO])
                        else:
                            nc.vector.tensor_copy(blk_sb, ps_y[:, bi * OO : (bi + 1) * OO])
                        if ge("tb"):
                            jb = junk_b[:, b * OO : (b + 1) * OO]
                            nc.vector.scalar_tensor_tensor(
                                jb.rearrange("p (a c) -> p a c", a=O),
                                blk_sb.rearrange("p (a c) -> p a c", a=O),
                                1.0,
                                blk_sb.rearrange("p (a c) -> p c a", a=O, c=O),
                                mult,
                                mult,
                                accum_out=tb[:, b : b + 1],
                            )
                if KSTAGE == "y":
                    target = work.tile([128, BC], f32, name="dbg_y")
                    nc.vector.tensor_copy(target, junk_b[:, 0:BC])

            if ge("tb"):
                pass
                if KSTAGE == "tb":
                    target = tb

            if ge("red"):
                # ---------------- per-sample scalars, sample-major [BC, .] ----------------
                # col0 = 2*(termA+termB) = NF^2, col1 = |v|^2  via lhsT=stacked tiles
                # |v| early (off critical path): vn1 = sqrt(|v|^2) + 1e-6
                vsq = work.tile([128, BC], f32)
                nc.vector.tensor_mul(vsq, vt, vt)
                ps_vn = ps.tile([BC, 1], f32, tag="red", bufs=1)
                nc.tensor.matmul(ps_vn, vsq, ones_c, start=True, stop=True, skip_group_check=True)
                vn1 = work.tile([BC, 1], f32)
                nc.scalar.activation(vn1, ps_vn, AF.Sqrt)
                nc.vector.tensor_scalar(vn1, vn1, 1e-6, -2.0, add, mult)
                # NF at the end: nf = sqrt(2*(termA+termB)); den = (nf+1e-6)*vn1
                ps_r4 = ps.tile([BC, 1], f32, tag="red", bufs=1)
                nc.tensor.matmul(ps_r4, ta, twos_c[0:O], start=True, stop=False, skip_group_check=True)
                nc.tensor.matmul(ps_r4, tb, twos_c, start=False, stop=True, skip_group_check=True)
                nf = work.tile([BC, 1], f32)
                nc.scalar.activation(nf, ps_r4, AF.Sqrt)
                den = work.tile([BC, 1], f32)
                nc.vector.scalar_tensor_tensor(den, nf, 1e-6, vn1, add, mult)
                rsc4 = work.tile([BC, 1], f32)
                nc.vector.reciprocal(rsc4, den)

                if KSTAGE == "red":
                    target = work.tile([128, BC], f32, name="dbg_red")
                    nc.vector.memset(target, 0.0)
                    nc.vector.tensor_copy(target[0:BC, 0:1], rsc4)

            if ge("full"):
                # transpose T -> [BC, 128] early (overlaps with norm pipeline),
                # transpose row-scale [1,BC] -> [BC,1], then
                # out = T_t * rsc4 - 0.1*dev  (dev rows already sample-major in xc_sb)
                ps_Tt = ps.tile([BC, 128], f32, tag="ps", bufs=2)
                nc.tensor.transpose(ps_Tt, ps_T_sb, id_sb)
                out_sb = work.tile([BC, 128], f32)
                nc.vector.scalar_tensor_tensor(
                    out_sb, ps_Tt, rsc4, devneg, mult, add
                )
                nc.sync.dma_start(out=acc_d[:], in_=out_sb)
            else:
                # ---------------- debug epilogue: transpose [pt, BC] -> [BC, pt] ----------------
                pt = target.shape[0]
                ps_out = ps.tile([BC, 128], f32, tag="ps", bufs=2)
                nc.tensor.transpose(ps_out[:, 0:pt], target, id_sb[:pt, :pt])
                out_sb = work.tile([BC, 128], f32)
                if pt < 128:
                    nc.vector.memset(out_sb, 0.0)
                nc.vector.tensor_copy(out_sb[:, 0:pt], ps_out[:, 0:pt])
                nc.sync.dma_start(out=acc_d[:], in_=out_sb)

    nc.finalize()
    return nc


def _get_program():
    global _PROGRAM
    if _PROGRAM is None:
        _PROGRAM = _build_program()
    return _PROGRAM


def make_in_maps(t, state_batch, x0, x1, W1, b1, W2):
    dev = state_batch[:B]
    v = state_batch[B:]
    w1_arr = np.ascontiguousarray(np.asarray(W1, np.float32))
    aux = np.empty((128, 23), np.float32)
    aux[:, 0:2] = np.asarray(b1, np.float32).reshape(2, 128).T
    aux[:, 2:22] = (
        np.asarray(W2, np.float32).reshape(2, 128, O).transpose(1, 0, 2).reshape(128, 2 * O)
    )
    aux[:, 22] = np.float32(np.asarray(t).ravel()[0])
    in_maps = []
    for c in range(NCORES):
        sl = slice(c * BC, (c + 1) * BC)
        xc = np.ascontiguousarray(
            np.concatenate([dev[sl], x0[sl], x1[sl], v[sl]], axis=0).astype(np.float32)
        )
        in_maps.append({"xc": xc, "aux": aux, "W1": w1_arr})
    return in_maps


def kernel(t, state_batch, x0, x1, W1, b1, W2, b2):
    from concourse import bass_utils

    t = np.asarray(t)
    state_batch = np.asarray(state_batch)
    x0 = np.asarray(x0)
    x1 = np.asarray(x1)
    W1 = np.asarray(W1)
    b1 = np.asarray(b1)
    W2 = np.asarray(W2)

    nc = _get_program()
    in_maps = make_in_maps(t, state_batch, x0, x1, W1, b1, W2)
    res = bass_utils.run_bass_kernel_spmd(nc, in_maps, core_ids=list(range(NCORES)))
    acc = np.concatenate([res.results[c]["acc"] for c in range(NCORES)], axis=0)
    v = state_batch[B:].astype(np.float32)
    return np.concatenate([v, acc.astype(np.float32)], axis=0)

